# revision 15
# baseline (speedup 1.0000x reference)
"""Trainium2 Bass kernel for nn_Net_24429773979977 (dense_mlp).

Computes: 3-layer MLP over [B,T,D]=[2048,128,128] -> f [N,64], row-normalize
u = f/max(||f||,eps), return (||sum u||^2 - sum|u|^2) / (2N).

Strategy (data-parallel over 8 cores, 32768 rows each):
 - Host pre-transposes x to feature-major xT [128, N] and casts to bf16.
 - Device per 512-row tile: L1/L2/L3 matmuls keep features on partitions;
   relu+bias fused into PSUM->SBUF evictions (DVE tensor_scalar / ACT
   activation). Row norms via an all-ones block-diag matmul on squared f,
   which also broadcasts nsq to all partitions; pairs of tiles are packed
   into 128 partitions for the 64-wide tail ops. u is produced by one DVE
   scalar_tensor_tensor (f+b3)/n with accum_out giving per-feature row-sums.
 - Host combines per-core partial sums (S) and nsq to the final scalar.
"""

import os
from contextlib import ExitStack

import numpy as np

B, T, D = 2048, 128, 128
N = B * T
NCORES = 8
NC_ROWS = N // NCORES          # 32768 rows per core
TILE = 512                     # rows per matmul tile (PSUM bank = 512 fp32)
PAIR_ROWS = 2 * TILE           # two tiles packed into 128 partitions
NPAIRS = NC_ROWS // PAIR_ROWS  # 32
H1, H2, H3 = 96, 72, 64
EPS = 1e-8
ARSQRT_FUNC = "Abs_reciprocal_sqrt"  # test_sim swaps to "Rsqrt" (same math; sim support)


def build_nc():
    import concourse.tile as tile
    from concourse import bacc, mybir

    f32 = mybir.dt.float32
    bf16 = mybir.dt.bfloat16

    nc = bacc.Bacc("TRN2", target_bir_lowering=False, debug=False)

    xT = nc.declare_dram_parameter("xT", [D, NC_ROWS], bf16, isOutput=False)
    w1t = nc.declare_dram_parameter("w1t", [D, H1], bf16, isOutput=False)
    w2t = nc.declare_dram_parameter("w2t", [H1, H2], bf16, isOutput=False)
    w3t = nc.declare_dram_parameter("w3t", [H2, H3], bf16, isOutput=False)
    onesbd = nc.declare_dram_parameter("onesbd", [128, 128], bf16, isOutput=False)
    b1 = nc.declare_dram_parameter("b1", [H1, 1], f32, isOutput=False)
    b2 = nc.declare_dram_parameter("b2", [H2, 1], f32, isOutput=False)
    b3s = nc.declare_dram_parameter("b3s", [128, 1], f32, isOutput=False)
    epsv = nc.declare_dram_parameter("epsv", [128, 1], f32, isOutput=False)

    s_out = nc.declare_dram_parameter("s_out", [128, NPAIRS], f32, isOutput=True)
    nsq_out = nc.declare_dram_parameter("nsq_out", [NPAIRS, 2, TILE], f32, isOutput=True)

    with tile.TileContext(nc) as tc, ExitStack() as ctx:
        consts = ctx.enter_context(tc.tile_pool(name="consts", bufs=1))
        xpool = ctx.enter_context(tc.tile_pool(name="x", bufs=4))
        h1pool = ctx.enter_context(tc.tile_pool(name="h1", bufs=3))
        h2pool = ctx.enter_context(tc.tile_pool(name="h2", bufs=3))
        fsqpool = ctx.enter_context(tc.tile_pool(name="fsq", bufs=2))
        nbpool = ctx.enter_context(tc.tile_pool(name="nb", bufs=2))
        upool = ctx.enter_context(tc.tile_pool(name="u", bufs=2))
        scolpool = ctx.enter_context(tc.tile_pool(name="scol", bufs=1))
        ps1 = ctx.enter_context(tc.tile_pool(name="ps1", bufs=2, space="PSUM"))
        ps2 = ctx.enter_context(tc.tile_pool(name="ps2", bufs=2, space="PSUM"))
        ps3 = ctx.enter_context(tc.tile_pool(name="ps3", bufs=2, space="PSUM"))
        psn = ctx.enter_context(tc.tile_pool(name="psn", bufs=2, space="PSUM"))

        w1_sb = consts.tile([D, H1], bf16, tag="w1")
        nc.sync.dma_start(out=w1_sb[:], in_=w1t[:])
        w2_sb = consts.tile([H1, H2], bf16, tag="w2")
        nc.sync.dma_start(out=w2_sb[:], in_=w2t[:])
        w3_sb = consts.tile([H2, H3], bf16, tag="w3")
        nc.sync.dma_start(out=w3_sb[:], in_=w3t[:])
        ones_sb = consts.tile([128, 128], bf16, tag="ones")
        nc.sync.dma_start(out=ones_sb[:], in_=onesbd[:])
        b1_sb = consts.tile([H1, 1], f32, tag="b1")
        nc.sync.dma_start(out=b1_sb[:], in_=b1[:])
        b2_sb = consts.tile([H2, 1], f32, tag="b2")
        nc.sync.dma_start(out=b2_sb[:], in_=b2[:])
        b3_sb = consts.tile([128, 1], f32, tag="b3")
        nc.sync.dma_start(out=b3_sb[:], in_=b3s[:])
        eps_sb = consts.tile([128, 1], f32, tag="epsv")
        nc.sync.dma_start(out=eps_sb[:], in_=epsv[:])

        scol = scolpool.tile([128, NPAIRS], f32, tag="scol")

        for p in range(NPAIRS):
            xt = xpool.tile([D, PAIR_ROWS], bf16, tag="xt")
            nc.sync.dma_start(
                out=xt[:], in_=xT[:, p * PAIR_ROWS:(p + 1) * PAIR_ROWS]
            )

            # layer 1 for both halves (W1 stationary reused)
            p1a = ps1.tile([H1, TILE], f32, tag="ps1")
            nc.tensor.matmul(p1a[:], w1_sb[:], xt[:, 0:TILE], start=True, stop=True)
            p1b = ps1.tile([H1, TILE], f32, tag="ps1")
            nc.tensor.matmul(p1b[:], w1_sb[:], xt[:, TILE:PAIR_ROWS], start=True, stop=True)

            # relu1 on DVE: h1 = max(psum1 + b1, 0)
            h1a = h1pool.tile([H1, TILE], bf16, tag="h1")
            nc.vector.tensor_scalar(h1a[:], p1a[:], b1_sb[:], 0.0,
                                    op0=mybir.AluOpType.add, op1=mybir.AluOpType.max)
            h1b = h1pool.tile([H1, TILE], bf16, tag="h1")
            nc.vector.tensor_scalar(h1b[:], p1b[:], b1_sb[:], 0.0,
                                    op0=mybir.AluOpType.add, op1=mybir.AluOpType.max)

            # layer 2
            p2a = ps2.tile([H2, TILE], f32, tag="ps2")
            nc.tensor.matmul(p2a[:], w2_sb[:], h1a[:], start=True, stop=True)
            p2b = ps2.tile([H2, TILE], f32, tag="ps2")
            nc.tensor.matmul(p2b[:], w2_sb[:], h1b[:], start=True, stop=True)

            # relu2 on ACT: h2 = relu(psum2 + b2)
            h2a = h2pool.tile([H2, TILE], bf16, tag="h2")
            nc.scalar.activation(h2a[:], p2a[:], mybir.ActivationFunctionType.Relu,
                                 bias=b2_sb[:], scale=1.0)
            h2b = h2pool.tile([H2, TILE], bf16, tag="h2")
            nc.scalar.activation(h2b[:], p2b[:], mybir.ActivationFunctionType.Relu,
                                 bias=b2_sb[:], scale=1.0)

            # layer 3: two tiles packed into one [128, TILE] PSUM bank
            p3 = ps3.tile([128, TILE], f32, tag="ps3")
            nc.tensor.matmul(p3[0:H3, :], w3_sb[:], h2a[:], start=True, stop=True)
            nc.tensor.matmul(p3[H3:128, :], w3_sb[:], h2b[:], start=True, stop=True)

            # fsq = square(psum3 + b3) -> bf16
            fsq = fsqpool.tile([128, TILE], bf16, tag="fsq")
            nc.scalar.activation(fsq[:], p3[:], mybir.ActivationFunctionType.Square,
                                 bias=b3_sb[:], scale=1.0)

            # nsq broadcast to both 64-partition halves via block-diag ones
            pn = psn.tile([128, TILE], f32, tag="psn")
            nc.tensor.matmul(pn[:], ones_sb[:], fsq[:], start=True, stop=True)

            # w = 1/sqrt(nsq + eps^2)
            nb = nbpool.tile([128, TILE], f32, tag="nb")
            arsqrt = getattr(mybir.ActivationFunctionType, ARSQRT_FUNC)
            nc.scalar.activation(nb[:], pn[:], arsqrt, bias=eps_sb[:], scale=1.0)

            # u = (psum3 + b3) * w ; accum_out = per-partition row sums
            u = upool.tile([128, TILE], bf16, tag="u")
            nc.vector.scalar_tensor_tensor(
                u[:], p3[:], b3_sb[:], nb[:],
                op0=mybir.AluOpType.add, op1=mybir.AluOpType.mult,
                accum_out=scol[:, p:p + 1],
            )

            # export w rows (one row per packed half) for host-side sum(u*u)
            nc.gpsimd.dma_start(out=nsq_out[p, 0], in_=nb[0:1, :])
            nc.gpsimd.dma_start(out=nsq_out[p, 1], in_=nb[H3:H3 + 1, :])

        nc.sync.dma_start(out=s_out[:], in_=scol[:])

    nc.compile()
    return nc


def _prep_host_inputs(x, W1, b1, W2, b2, W3, b3):
    import ml_dtypes

    bf = ml_dtypes.bfloat16
    xflat = np.ascontiguousarray(x.reshape(N, D))
    in_maps = []
    w1t = np.ascontiguousarray(W1.T).astype(bf)
    w2t = np.ascontiguousarray(W2.T).astype(bf)
    w3t = np.ascontiguousarray(W3.T).astype(bf)
    onesbd = np.zeros((128, 128), np.float32)
    onesbd[:H3, :H3] = 1.0
    onesbd[H3:, H3:] = 1.0
    onesbd = onesbd.astype(bf)
    b1c = np.ascontiguousarray(b1.reshape(H1, 1), dtype=np.float32)
    b2c = np.ascontiguousarray(b2.reshape(H2, 1), dtype=np.float32)
    b3s = np.concatenate([b3, b3]).reshape(128, 1).astype(np.float32)
    for c in range(NCORES):
        xT_c = np.ascontiguousarray(
            xflat[c * NC_ROWS:(c + 1) * NC_ROWS].T
        ).astype(bf)
        in_maps.append({
            "xT": xT_c, "w1t": w1t, "w2t": w2t, "w3t": w3t,
            "onesbd": onesbd, "b1": b1c, "b2": b2c, "b3s": b3s,
            "epsv": np.full((128, 1), EPS * EPS, np.float32),
        })
    return in_maps


def _combine(results):
    """results: list of per-core dicts with s_out [128, NPAIRS], nsq_out."""
    S = np.zeros(H3, np.float64)
    usq = 0.0
    for r in results:
        sc = np.asarray(r["s_out"], np.float64)
        S += sc[:H3].sum(axis=1) + sc[H3:128].sum(axis=1)
        w = np.asarray(r["nsq_out"], np.float64).ravel()  # 1/sqrt(nsq + eps^2)
        nsq = np.maximum(1.0 / (w * w) - EPS * EPS, 0.0)
        usq += float(np.sum(nsq * w * w))
    pair = 0.5 * (S @ S - usq)
    return np.float32(pair / N)


_NC_CACHE = {}


def kernel(x, W1, b1, W2, b2, W3, b3):
    from concourse.bass_utils import run_bass_kernel_spmd

    if "nc" not in _NC_CACHE:
        _NC_CACHE["nc"] = build_nc()
    nc = _NC_CACHE["nc"]
    in_maps = _prep_host_inputs(
        np.asarray(x, np.float32), np.asarray(W1, np.float32),
        np.asarray(b1, np.float32), np.asarray(W2, np.float32),
        np.asarray(b2, np.float32), np.asarray(W3, np.float32),
        np.asarray(b3, np.float32),
    )
    res = run_bass_kernel_spmd(nc, in_maps, list(range(NCORES)))
    return _combine(res.results)


if __name__ == "__main__":
    pass
